# revision 24
# baseline (speedup 1.0000x reference)
"""Trainium2 Bass kernel for nn_Encoder_61177514164477 (meta-GCN LSTM encoder).

Sharding: 8 cores = 4 batch groups x 2 node-halves. Core c handles batch
b = c//2 and node rows [half*1024, (half+1)*1024) with half = c%2.
G^T (bf16) stays SBUF-resident per core; one pairwise masked ReduceScatter
per timestep exchanges h states between the two halves of each batch pair.

Restructured dataflow: only ONE einsum1 per step,
P_t = G^T . [h0_t | h1_{t-1}] (128 stationary rows). Layer-1 einsum2 at
step t uses all 128 rows of P_t; layer-0 einsum2 at step t+1 reuses rows
0:64 (G.h0_t). The G.x_t terms (x is known upfront) and G.h0_init are
precomputed on the HOST, as are the tiny meta-learner MLPs. The layer-0
x-part + bias ride a 64-row host-built stationary (gxT: 6 data rows, a
ones row for the bias, zero padding).
"""
import numpy as np
import ml_dtypes

import concourse.bass as bass
import concourse.mybir as mybir
import concourse.tile as tile
import concourse.bacc as bacc
import concourse.tile_utils as tile_utils
from concourse.bass_utils import run_bass_kernel_spmd

tile_utils.max_sbuf_usage = 204 * 1024

L, B, T, N, C, H, K, M = 2, 4, 8, 2048, 2, 64, 3, 32
DIN0, DIN1, DOUT = C + H, 2 * H, 4 * H
HALF = N // 2          # 1024 rows per core
JT = N // 128          # 16 j-tiles (local order: 8 own + 8 partner)
IT = HALF // 128       # 8 own i-tiles
NCORES = 8
PAIRS = [[0, 1], [2, 3], [4, 5], [6, 7]]
XR = 2 * K + 1         # used rows of gxT/w0x: (k,c) pairs + ones row

F32 = mybir.dt.float32
BF16 = mybir.dt.bfloat16
BF = ml_dtypes.bfloat16

_CACHE = {}
LAST_RESULT = None


def _build():
    if "nc" in _CACHE:
        return _CACHE["nc"]
    nc = bacc.Bacc(None, target_bir_lowering=False, debug=False)

    gt_in = nc.declare_dram_parameter("gt", [K, JT, 128, HALF], BF16, isOutput=False)
    s0h_in = nc.declare_dram_parameter("sup0h", [K, 64, HALF], BF16, isOutput=False)
    w0h_in = nc.declare_dram_parameter("w0h", [64, T * K * DOUT], BF16, isOutput=False)
    w0x_in = nc.declare_dram_parameter("w0x", [64, T * DOUT], BF16, isOutput=False)
    gxt_in = nc.declare_dram_parameter("gxt", [64, T * HALF], BF16, isOutput=False)
    w1_in = nc.declare_dram_parameter("w1", [DIN1, T * K * DOUT], BF16, isOutput=False)
    b1_in = nc.declare_dram_parameter("bias1", [128, T * DOUT], BF16, isOutput=False)
    c0_in = nc.declare_dram_parameter("c0_init", [128, IT * H], F32, isOutput=False)
    c1_in = nc.declare_dram_parameter("c1_init", [128, IT * H], F32, isOutput=False)
    h1b_in = nc.declare_dram_parameter("h1b_init", [128, IT * H], BF16, isOutput=False)
    mask_in = nc.declare_dram_parameter("mask", [128, 2], F32, isOutput=False)
    out_ext = nc.declare_dram_parameter("out", [2, L, 128, IT * H], F32, isOutput=True)

    MULT = mybir.AluOpType.mult
    ADD = mybir.AluOpType.add
    SIG = mybir.ActivationFunctionType.Sigmoid
    TANH = mybir.ActivationFunctionType.Tanh

    with tile.TileContext(nc) as tc:
        with tc.tile_pool(name="const", bufs=1) as cpool, \
             tc.tile_pool(name="stat", bufs=2) as spool, \
             tc.tile_pool(name="work", bufs=1) as wpool, \
             tc.tile_pool(name="psum", bufs=1, space="PSUM") as ppool, \
             tc.tile_pool(name="dram", bufs=1, space="DRAM") as dpool:

            # ---- host-prepped constants (DMA order = priority) ----
            s0h_sb = []
            for k in range(K):
                t_ = cpool.tile([64, HALF], BF16, name=f"s0h{k}", tag=f"s0h{k}")
                nc.sync.dma_start(t_[:], s0h_in[k])
                s0h_sb.append(t_)
            w0h_sb = cpool.tile([64, T * K * DOUT], BF16, name="w0h", tag="w0h")
            nc.sync.dma_start(w0h_sb[:], w0h_in[:])
            w0x_sb = cpool.tile([64, T * DOUT], BF16, name="w0x", tag="w0x")
            nc.sync.dma_start(w0x_sb[:], w0x_in[:])
            gxt_sb = cpool.tile([64, T * HALF], BF16, name="gxt", tag="gxt")
            nc.sync.dma_start(gxt_sb[:], gxt_in[:])
            c_all = []
            for l, cin in ((0, c0_in), (1, c1_in)):
                ct = cpool.tile([128, IT * H], F32, name=f"c{l}_all", tag=f"c{l}_all")
                nc.sync.dma_start(ct[:], cin[:])
                c_all.append(ct)
            h1i_sb = cpool.tile([128, IT * H], BF16, name="h1i_sb", tag="h1i_sb")
            nc.sync.dma_start(h1i_sb[:], h1b_in[:])
            mask_sb = cpool.tile([128, 2], F32, name="mask_sb", tag="mask_sb")
            nc.sync.dma_start(mask_sb[:], mask_in[:])
            mk = [mask_sb[:, 0:1], mask_sb[:, 1:2]]

            # dram bounce/output buffers: ONE combined ReduceScatter per
            # step carries [h0_t | h1_(t-1)] so the CC stream is paid once
            bounce = [dpool.tile([2, 2, 128, IT * H], BF16, name=f"bounce{i}",
                                 tag=f"bounce{i}") for i in range(2)]
            rs_out = [dpool.tile([2, 128, IT * H], BF16, name=f"rso{i}",
                                 tag=f"rso{i}") for i in range(2)]

            def send_h(src_ap, tslot, l):
                """bounce[tslot][s][l] <- own h * mask_s for both slots s."""
                for sslot in range(2):
                    hm = wpool.tile([128, IT * H], BF16, name="hm",
                                    tag=f"hm{l}{sslot}", bufs=1)
                    nc.vector.tensor_scalar_mul(
                        hm[:].rearrange("p (it c) -> p it c", c=H), src_ap,
                        mk[sslot])
                    eng = nc.sync if sslot == 0 else nc.scalar
                    eng.dma_start(bounce[tslot][sslot, l], hm[:])

            def rs_fire(tslot):
                nc.gpsimd.collective_compute(
                    "ReduceScatter", mybir.AluOpType.add, replica_groups=PAIRS,
                    ins=[bounce[tslot].opt()], outs=[rs_out[tslot].opt()],
                )

            # warmup collective: absorbs the CC-init barrier + first-op cost
            # while the gt DMAs stream in (result unused)
            dum_in = dpool.tile([2, 128, 2], F32, name="dum_in", tag="dum_in")
            dum_out = dpool.tile([128, 2], F32, name="dum_out", tag="dum_out")
            for s in range(2):
                nc.sync.dma_start(dum_in[s], mask_sb[:])
            nc.gpsimd.collective_compute(
                "ReduceScatter", mybir.AluOpType.add, replica_groups=PAIRS,
                ins=[dum_in.opt()], outs=[dum_out.opt()],
            )

            # t=0: masked init-h1 into bounce[0] slot l=1
            send_h(h1i_sb[:].rearrange("p (it c) -> p it c", c=H), 0, 1)

            # ---- G^T tiles: own j-half first (e1 own chases these DMAs) ----
            gt_sb = []
            for jt in range(JT):
                t_ = cpool.tile([128, K * HALF], BF16, name=f"gt{jt}", tag=f"gt{jt}")
                src = gt_in[:, jt, :, :].rearrange("k p i -> p k i")
                nc.sync.dma_start(t_[:].rearrange("p (k i) -> p k i", k=K), src)
                gt_sb.append(t_)

            w1_sb = cpool.tile([DIN1, T * K * DOUT], BF16, name="w1_sb", tag="w1_sb")
            nc.sync.dma_start(w1_sb[:], w1_in[:])
            b1_sb = cpool.tile([128, T * DOUT], BF16, name="b1_sb", tag="b1_sb")
            nc.sync.dma_start(b1_sb[:], b1_in[:])

            def e2_l0(t, supP_h, conv0):
                """conv0[:, it] = sum_k supP_h[k][0:64].T @ w0h + gxT.T @ w0x."""
                for ih in range(2):
                    for it in range(ih * 4, ih * 4 + 4):
                        pc = ppool.tile([128, DOUT], F32, name="e2p", tag="e2p",
                                        bufs=2)
                        for k in range(K):
                            nc.tensor.matmul(
                                pc[:],
                                supP_h[k][0:64, it * 128:(it + 1) * 128],
                                w0h_sb[:, (t * K + k) * DOUT:(t * K + k + 1) * DOUT],
                                start=(k == 0), stop=False,
                            )
                        nc.tensor.matmul(
                            pc[:],
                            gxt_sb[:, t * HALF + it * 128: t * HALF + (it + 1) * 128],
                            w0x_sb[:, t * DOUT:(t + 1) * DOUT],
                            start=False, stop=True,
                        )
                        dst = conv0[:, it * DOUT:(it + 1) * DOUT]
                        if it % 2 == 0:
                            nc.vector.tensor_copy(dst, pc[:])
                        else:
                            nc.scalar.copy(dst, pc[:])
                    yield ih

            def gates(conv_all, ih, c_t, h_dst):
                """LSTM gates on half ih: conv [128, 4it x 4gates x 64]."""
                HB = 4 * H
                cv = conv_all[:, ih * 4 * DOUT:(ih + 1) * 4 * DOUT].rearrange(
                    "p (it g c) -> p it g c", g=4, c=H)
                sig_i = wpool.tile([128, HB], BF16, name="g_si", tag="g_si", bufs=2)
                sig_f = wpool.tile([128, HB], BF16, name="g_sf", tag="g_sf", bufs=2)
                sig_o = wpool.tile([128, HB], BF16, name="g_so", tag="g_so", bufs=2)
                tanh_g = wpool.tile([128, HB], BF16, name="g_tg", tag="g_tg", bufs=2)
                nc.scalar.activation(sig_f[:], cv[:, :, 1, :], SIG)
                nc.scalar.activation(sig_i[:], cv[:, :, 0, :], SIG)
                nc.scalar.activation(tanh_g[:], cv[:, :, 3, :], TANH)
                nc.scalar.activation(sig_o[:], cv[:, :, 2, :], SIG)
                m1 = wpool.tile([128, HB], F32, name="g_m1", tag="g_m1", bufs=1)
                m2 = wpool.tile([128, HB], F32, name="g_m2", tag="g_m2", bufs=1)
                ch = c_t[:, ih * HB:(ih + 1) * HB]
                nc.vector.tensor_tensor(m1[:], sig_f[:], ch, MULT)
                nc.vector.tensor_tensor(m2[:], sig_i[:], tanh_g[:], MULT)
                nc.vector.tensor_tensor(ch, m1[:], m2[:], ADD)
                tanh_c = wpool.tile([128, HB], BF16, name="g_tc", tag="g_tc", bufs=2)
                nc.scalar.activation(tanh_c[:], ch, TANH)
                nc.vector.tensor_tensor(h_dst, sig_o[:], tanh_c[:], MULT)

            hf1 = wpool.tile([128, IT * H], F32, name="hf1", tag="hf1")
            supP_prev = s0h_sb           # t=-1: host G.h0_init (64 rows)
            stat_cur = spool.tile([128, JT * DIN1], BF16, name="stat1", tag="stat1")
            s1v = stat_cur[:].rearrange("p (jt c) -> p jt c", c=DIN1)
            nc.vector.tensor_copy(
                s1v[:, 0:8, H:DIN1],
                h1i_sb[:].rearrange("p (it c) -> p it c", c=H))

            def e1_mm(psumP, jt, ih, start, stop):
                lhs = s1v[:, jt, :]
                for k in range(K):
                    nc.tensor.matmul(
                        psumP[k][ih][:],
                        lhs,
                        gt_sb[jt][:, k * HALF + ih * 512:
                                  k * HALF + ih * 512 + 512],
                        start=start, stop=stop,
                    )

            def e2_l1_its(t, supP, conv1, its):
                for it in its:
                    pc = ppool.tile([128, DOUT], F32, name="e2p", tag="e2p",
                                    bufs=2)
                    for k in range(K):
                        nc.tensor.matmul(
                            pc[:],
                            supP[k][:, it * 128:(it + 1) * 128],
                            w1_sb[:, (t * K + k) * DOUT:(t * K + k + 1) * DOUT],
                            start=(k == 0), stop=(k == K - 1),
                        )
                    dst = conv1[:, it * DOUT:(it + 1) * DOUT]
                    nc.vector.tensor_tensor(
                        dst, pc[:], b1_sb[:, t * DOUT:(t + 1) * DOUT], ADD)

            for t in range(T):
                # ---------------- layer 0: einsum2 + gates ----------------
                conv0 = wpool.tile([128, IT * DOUT], BF16, name="conv0", tag="conv0")
                for ih in e2_l0(t, supP_prev, conv0):
                    gates(conv0, ih, c_all[0], s1v[:, ih * 4:(ih + 1) * 4, 0:H])
                send_h(s1v[:, 0:8, 0:H], t % 2, 0)
                rs_fire(t % 2)

                # ---------------- einsum1: P_t = G^T [h0_t | h1_{t-1}] ----
                psumP = [[ppool.tile([128, 512], F32, name=f"e1p{k}{ih}",
                                     tag=f"e1p{k}{ih}", bufs=1)
                          for ih in range(2)] for k in range(K)]
                for ih in range(2):
                    for jt in range(8):
                        e1_mm(psumP, jt, ih, jt == 0, False)
                # partner halves arrive at static offsets
                nc.sync.dma_start(
                    s1v[:, 8:16, 0:H],
                    rs_out[t % 2][0].rearrange("p (it c) -> p it c", c=H))
                nc.scalar.dma_start(
                    s1v[:, 8:16, H:DIN1],
                    rs_out[t % 2][1].rearrange("p (it c) -> p it c", c=H))
                supP = [wpool.tile([128, HALF], BF16, name=f"supP{k}",
                                   tag=f"supP{k}", bufs=2) for k in range(K)]
                # partner ih0, evac ih0 (overlaps partner ih1 on tensor)
                for jt in range(8, 16):
                    e1_mm(psumP, jt, 0, False, jt == 15)
                for k in range(K):
                    dst = supP[k][:, 0:512]
                    if k % 2 == 0:
                        nc.vector.tensor_copy(dst, psumP[k][0][:])
                    else:
                        nc.scalar.copy(dst, psumP[k][0][:])
                for jt in range(8, 16):
                    e1_mm(psumP, jt, 1, False, jt == 15)

                # ---------------- layer 1: einsum2 + gates ----------------
                conv1 = wpool.tile([128, IT * DOUT], BF16, name="conv1", tag="conv1")
                e2_l1_its(t, supP, conv1, range(0, 4))
                for k in range(K):
                    dst = supP[k][:, 512:1024]
                    if k % 2 == 0:
                        nc.scalar.copy(dst, psumP[k][1][:])
                    else:
                        nc.vector.tensor_copy(dst, psumP[k][1][:])
                e2_l1_its(t, supP, conv1, range(4, 8))
                if t + 1 < T:
                    stat_next = spool.tile([128, JT * DIN1], BF16, name="stat1",
                                           tag="stat1")
                    s1v_next = stat_next[:].rearrange("p (jt c) -> p jt c", c=DIN1)
                    h1_dst = lambda ih: s1v_next[:, ih * 4:(ih + 1) * 4, H:DIN1]
                else:
                    h1_dst = lambda ih: hf1[:].rearrange(
                        "p (it c) -> p it c", c=H)[:, ih * 4:(ih + 1) * 4, :]
                for ih in range(2):
                    gates(conv1, ih, c_all[1], h1_dst(ih))
                if t + 1 < T:
                    send_h(s1v_next[:, 0:8, H:DIN1], (t + 1) % 2, 1)
                    s1v = s1v_next
                supP_prev = supP

            # ---------------- outputs ----------------
            hf0 = wpool.tile([128, IT * H], F32, name="hf0", tag="hf0")
            nc.vector.tensor_copy(
                hf0[:].rearrange("p (it c) -> p it c", c=H), s1v[:, 0:8, 0:H])
            nc.sync.dma_start(out_ext[0, 0], hf0[:])
            nc.sync.dma_start(out_ext[0, 1], hf1[:])
            nc.sync.dma_start(out_ext[1, 0], c_all[0][:])
            nc.sync.dma_start(out_ext[1, 1], c_all[1][:])

    nc.compile()
    _CACHE["nc"] = nc
    return nc


def _host_prep(inputs):
    """Per-core input maps (all device layouts built here)."""
    G = np.asarray(inputs["G"], np.float32)
    x_seq = np.asarray(inputs["x_seq"], np.float32)
    init_h = np.asarray(inputs["init_h"], np.float32)
    init_c = np.asarray(inputs["init_c"], np.float32)
    x_meta = np.asarray(inputs["x_meta"], np.float32)

    def mlp(b, w1, b1, w2, b2):
        hid = np.maximum(x_meta[b] @ w1 + b1, 0.0)
        return hid @ w2 + b2

    GF = G.reshape(K * N, N)
    in_maps = []
    for c in range(NCORES):
        b, half = c // 2, c % 2
        own = np.arange(half * HALF, (half + 1) * HALF)
        par = np.arange((1 - half) * HALF, (2 - half) * HALF)
        jperm = np.concatenate([own, par])

        # GT[k, j_local, i_own] -> [K, JT, 128, HALF]
        gt = G[:, own, :].transpose(0, 2, 1)[:, jperm, :]
        gt = np.ascontiguousarray(gt.reshape(K, JT, 128, HALF)).astype(BF)

        # host Gx: gxT rows (k,c) at k*C+c, ones row XR-1, zero padding
        xb = np.ascontiguousarray(x_seq[b].transpose(1, 0, 2).reshape(N, T * C))
        gx = (GF @ xb).reshape(K, N, T, C)
        gxt = np.zeros((64, T * HALF), np.float32)
        for k in range(K):
            for cc in range(C):
                gxt[k * C + cc] = gx[k, own, :, cc].T.reshape(T * HALF)
        gxt[XR - 1] = 1.0
        gxt = gxt.astype(BF)

        # host G.h0_init (skip the matmul for the all-zeros init case)
        s0h = np.zeros((K, 64, HALF), np.float32)
        if init_h[0, b].any():
            gh = (GF @ init_h[0, b]).reshape(K, N, H)
            s0h = np.ascontiguousarray(gh[:, own, :].transpose(0, 2, 1))
        s0h = s0h.astype(BF)

        # layer-0 weights: W0 rows [x(0:C) | h(C:C+H)]
        W0 = mlp(b, inputs["lw1_0"], inputs["lb1_0"], inputs["lw2_0"], inputs["lb2_0"])
        W0 = np.asarray(W0, np.float32).reshape(T, K, DIN0, DOUT)
        bias0 = np.asarray(
            mlp(b, inputs["bw1_0"], inputs["bb1_0"], inputs["bw2_0"], inputs["bb2_0"]),
            np.float32)
        w0h = W0[:, :, C:, :].transpose(2, 0, 1, 3).reshape(64, T * K * DOUT).astype(BF)
        w0x = np.zeros((64, T * DOUT), np.float32)
        for k in range(K):
            for cc in range(C):
                w0x[k * C + cc] = W0[:, k, cc, :].reshape(T * DOUT)
        w0x[XR - 1] = bias0.reshape(T * DOUT)
        w0x = w0x.astype(BF)

        W1 = mlp(b, inputs["lw1_1"], inputs["lb1_1"], inputs["lw2_1"], inputs["lb2_1"])
        W1 = np.asarray(W1, np.float32).reshape(T, K, DIN1, DOUT)
        w1 = W1.transpose(2, 0, 1, 3).reshape(DIN1, T * K * DOUT).astype(BF)
        bias1 = np.asarray(
            mlp(b, inputs["bw1_1"], inputs["bb1_1"], inputs["bw2_1"], inputs["bb2_1"]),
            np.float32)
        b1 = np.ascontiguousarray(
            np.broadcast_to(bias1.reshape(1, T * DOUT), (128, T * DOUT))).astype(BF)

        c0 = np.ascontiguousarray(
            init_c[0, b][own].reshape(IT, 128, H).transpose(1, 0, 2).reshape(128, IT * H))
        c1 = np.ascontiguousarray(
            init_c[1, b][own].reshape(IT, 128, H).transpose(1, 0, 2).reshape(128, IT * H))
        h1b = init_h[1, b][own].reshape(IT, 128, H).transpose(1, 0, 2).reshape(
            128, IT * H).astype(BF)

        in_maps.append({
            "gt": gt,
            "sup0h": s0h,
            "w0h": np.ascontiguousarray(w0h),
            "w0x": np.ascontiguousarray(w0x),
            "gxt": np.ascontiguousarray(gxt),
            "w1": np.ascontiguousarray(w1),
            "bias1": b1,
            "c0_init": np.ascontiguousarray(c0, np.float32),
            "c1_init": np.ascontiguousarray(c1, np.float32),
            "h1b_init": np.ascontiguousarray(h1b),
            "mask": np.ascontiguousarray(np.broadcast_to(
                np.array([1 - half, half], np.float32).reshape(1, 2), (128, 2))),
        })
    return in_maps


def kernel(**inputs) -> np.ndarray:
    global LAST_RESULT
    nc = _build()
    in_maps = _host_prep(inputs)
    res = run_bass_kernel_spmd(nc, in_maps, list(range(NCORES)))
    LAST_RESULT = res

    out = np.zeros((2, L, B, N, H), np.float32)
    for c in range(NCORES):
        b, half = c // 2, c % 2
        o = res.results[c]["out"].reshape(2, L, 128, IT, H)
        out[:, :, b, half * HALF:(half + 1) * HALF, :] = o.transpose(0, 1, 3, 2, 4).reshape(
            2, L, HALF, H)
    return out


# revision 26
# speedup vs baseline: 1.1257x; 1.1257x over previous
"""Trainium2 Bass kernel for nn_Encoder_61177514164477 (meta-GCN LSTM encoder).

Sharding: 8 cores = 4 batch groups x 2 node-halves. Core c handles batch
b = c//2 and node rows [half*1024, (half+1)*1024) with half = c%2.
G^T (bf16) stays SBUF-resident per core; one pairwise masked ReduceScatter
per timestep exchanges h states between the two halves of each batch pair.

Restructured dataflow: only ONE einsum1 per step,
P_t = G^T . [h0_t | h1_{t-1}] (128 stationary rows). Layer-1 einsum2 at
step t uses all 128 rows of P_t; layer-0 einsum2 at step t+1 reuses rows
0:64 (G.h0_t). The G.x_t terms (x is known upfront) and G.h0_init are
precomputed on the HOST, as are the tiny meta-learner MLPs. The layer-0
x-part + bias ride a 64-row host-built stationary (gxT: 6 data rows, a
ones row for the bias, zero padding).
"""
import numpy as np
import ml_dtypes

import concourse.bass as bass
import concourse.mybir as mybir
import concourse.tile as tile
import concourse.bacc as bacc
import concourse.tile_utils as tile_utils
from concourse.bass_utils import run_bass_kernel_spmd

tile_utils.max_sbuf_usage = 204 * 1024

L, B, T, N, C, H, K, M = 2, 4, 8, 2048, 2, 64, 3, 32
DIN0, DIN1, DOUT = C + H, 2 * H, 4 * H
HALF = N // 2          # 1024 rows per core
JT = N // 128          # 16 j-tiles (local order: 8 own + 8 partner)
IT = HALF // 128       # 8 own i-tiles
NCORES = 8
PAIRS = [[0, 1], [2, 3], [4, 5], [6, 7]]
XR = 2 * K + 1         # used rows of gxT/w0x: (k,c) pairs + ones row

F32 = mybir.dt.float32
BF16 = mybir.dt.bfloat16
BF = ml_dtypes.bfloat16

_CACHE = {}
LAST_RESULT = None


def _build():
    if "nc" in _CACHE:
        return _CACHE["nc"]
    nc = bacc.Bacc(None, target_bir_lowering=False, debug=False)

    gt_in = nc.declare_dram_parameter("gt", [K, JT, 128, HALF], BF16, isOutput=False)
    s0h_in = nc.declare_dram_parameter("sup0h", [K, 64, HALF], BF16, isOutput=False)
    w0h_in = nc.declare_dram_parameter("w0h", [64, T * K * DOUT], BF16, isOutput=False)
    w0x_in = nc.declare_dram_parameter("w0x", [64, T * DOUT], BF16, isOutput=False)
    gxt_in = nc.declare_dram_parameter("gxt", [64, T * HALF], BF16, isOutput=False)
    w1_in = nc.declare_dram_parameter("w1", [DIN1, T * K * DOUT], BF16, isOutput=False)
    b1_in = nc.declare_dram_parameter("bias1", [128, T * DOUT], BF16, isOutput=False)
    c0_in = nc.declare_dram_parameter("c0_init", [128, IT * H], F32, isOutput=False)
    c1_in = nc.declare_dram_parameter("c1_init", [128, IT * H], F32, isOutput=False)
    h1b_in = nc.declare_dram_parameter("h1b_init", [128, IT * H], BF16, isOutput=False)
    mask_in = nc.declare_dram_parameter("mask", [128, 2], F32, isOutput=False)
    out_ext = nc.declare_dram_parameter("out", [2, L, 128, IT * H], F32, isOutput=True)

    MULT = mybir.AluOpType.mult
    ADD = mybir.AluOpType.add
    SIG = mybir.ActivationFunctionType.Sigmoid
    TANH = mybir.ActivationFunctionType.Tanh

    with tile.TileContext(nc) as tc:
        with tc.tile_pool(name="const", bufs=1) as cpool, \
             tc.tile_pool(name="stat", bufs=2) as spool, \
             tc.tile_pool(name="work", bufs=1) as wpool, \
             tc.tile_pool(name="psum", bufs=1, space="PSUM") as ppool, \
             tc.tile_pool(name="dram", bufs=1, space="DRAM") as dpool:

            # ---- host-prepped constants (DMA order = priority) ----
            s0h_sb = []
            for k in range(K):
                t_ = cpool.tile([64, HALF], BF16, name=f"s0h{k}", tag=f"s0h{k}")
                nc.sync.dma_start(t_[:], s0h_in[k])
                s0h_sb.append(t_)
            w0h_sb = cpool.tile([64, T * K * DOUT], BF16, name="w0h", tag="w0h")
            nc.sync.dma_start(w0h_sb[:], w0h_in[:])
            w0x_sb = cpool.tile([64, T * DOUT], BF16, name="w0x", tag="w0x")
            nc.sync.dma_start(w0x_sb[:], w0x_in[:])
            gxt_sb = cpool.tile([64, T * HALF], BF16, name="gxt", tag="gxt")
            nc.sync.dma_start(gxt_sb[:], gxt_in[:])
            c_all = []
            for l, cin in ((0, c0_in), (1, c1_in)):
                ct = cpool.tile([128, IT * H], F32, name=f"c{l}_all", tag=f"c{l}_all")
                nc.sync.dma_start(ct[:], cin[:])
                c_all.append(ct)
            h1i_sb = cpool.tile([128, IT * H], BF16, name="h1i_sb", tag="h1i_sb")
            nc.sync.dma_start(h1i_sb[:], h1b_in[:])
            mask_sb = cpool.tile([128, 2], F32, name="mask_sb", tag="mask_sb")
            nc.sync.dma_start(mask_sb[:], mask_in[:])
            mk = [mask_sb[:, 0:1], mask_sb[:, 1:2]]

            # dram bounce/output buffers: one pair per layer per parity so the
            # two per-step ReduceScatters are independent and fire early
            bounce = [[dpool.tile([2, 128, IT * H], BF16, name=f"bounce{l}{i}",
                                  tag=f"bounce{l}{i}") for i in range(2)]
                      for l in range(2)]
            rs_out = [[dpool.tile([128, IT * H], BF16, name=f"rso{l}{i}",
                                  tag=f"rso{l}{i}") for i in range(2)]
                      for l in range(2)]

            def send_h(src_ap, tslot, l):
                """bounce[l][tslot][s] <- own h * mask_s for both slots s."""
                for sslot in range(2):
                    hm = wpool.tile([128, IT * H], BF16, name="hm",
                                    tag=f"hm{l}{sslot}", bufs=1)
                    nc.vector.tensor_scalar_mul(
                        hm[:].rearrange("p (it c) -> p it c", c=H), src_ap,
                        mk[sslot])
                    nc.sync.dma_start(bounce[l][tslot][sslot], hm[:])

            def rs_fire(tslot, l):
                nc.gpsimd.collective_compute(
                    "ReduceScatter", mybir.AluOpType.add, replica_groups=PAIRS,
                    ins=[bounce[l][tslot].opt()], outs=[rs_out[l][tslot].opt()],
                )

            # t=0: masked init-h1 into bounce[1][0]; its RS fires right at
            # startup and absorbs the CC-init barrier + cold first-op cost
            # (its consumer is ~60us away)
            send_h(h1i_sb[:].rearrange("p (it c) -> p it c", c=H), 0, 1)
            rs_fire(0, 1)

            # ---- G^T tiles: own j-half first (e1 own chases these DMAs) ----
            gt_sb = []
            for jt in range(JT):
                t_ = cpool.tile([128, K * HALF], BF16, name=f"gt{jt}", tag=f"gt{jt}")
                src = gt_in[:, jt, :, :].rearrange("k p i -> p k i")
                nc.sync.dma_start(t_[:].rearrange("p (k i) -> p k i", k=K), src)
                gt_sb.append(t_)

            w1_sb = cpool.tile([DIN1, T * K * DOUT], BF16, name="w1_sb", tag="w1_sb")
            nc.sync.dma_start(w1_sb[:], w1_in[:])
            b1_sb = cpool.tile([128, T * DOUT], BF16, name="b1_sb", tag="b1_sb")
            nc.sync.dma_start(b1_sb[:], b1_in[:])

            def e2_l0(t, supP_h, conv0):
                """conv0[:, it] = sum_k supP_h[k][0:64].T @ w0h + gxT.T @ w0x."""
                for ih in range(2):
                    for it in range(ih * 4, ih * 4 + 4):
                        pc = ppool.tile([128, DOUT], F32, name="e2p", tag="e2p",
                                        bufs=2)
                        for k in range(K):
                            nc.tensor.matmul(
                                pc[:],
                                supP_h[k][0:64, it * 128:(it + 1) * 128],
                                w0h_sb[:, (t * K + k) * DOUT:(t * K + k + 1) * DOUT],
                                start=(k == 0), stop=False,
                            )
                        nc.tensor.matmul(
                            pc[:],
                            gxt_sb[:, t * HALF + it * 128: t * HALF + (it + 1) * 128],
                            w0x_sb[:, t * DOUT:(t + 1) * DOUT],
                            start=False, stop=True,
                        )
                        dst = conv0[:, it * DOUT:(it + 1) * DOUT]
                        if it % 2 == 0:
                            nc.vector.tensor_copy(dst, pc[:])
                        else:
                            nc.scalar.copy(dst, pc[:])
                    yield ih

            def gates(conv_all, ih, c_t, h_dst):
                """LSTM gates on half ih: conv [128, 4it x 4gates x 64]."""
                HB = 4 * H
                cv = conv_all[:, ih * 4 * DOUT:(ih + 1) * 4 * DOUT].rearrange(
                    "p (it g c) -> p it g c", g=4, c=H)
                sig_i = wpool.tile([128, HB], BF16, name="g_si", tag="g_si", bufs=2)
                sig_f = wpool.tile([128, HB], BF16, name="g_sf", tag="g_sf", bufs=2)
                sig_o = wpool.tile([128, HB], BF16, name="g_so", tag="g_so", bufs=2)
                tanh_g = wpool.tile([128, HB], BF16, name="g_tg", tag="g_tg", bufs=2)
                nc.scalar.activation(sig_f[:], cv[:, :, 1, :], SIG)
                nc.scalar.activation(sig_i[:], cv[:, :, 0, :], SIG)
                nc.scalar.activation(tanh_g[:], cv[:, :, 3, :], TANH)
                nc.scalar.activation(sig_o[:], cv[:, :, 2, :], SIG)
                m1 = wpool.tile([128, HB], F32, name="g_m1", tag="g_m1", bufs=1)
                m2 = wpool.tile([128, HB], F32, name="g_m2", tag="g_m2", bufs=1)
                ch = c_t[:, ih * HB:(ih + 1) * HB]
                nc.vector.tensor_tensor(m1[:], sig_f[:], ch, MULT)
                nc.vector.tensor_tensor(m2[:], sig_i[:], tanh_g[:], MULT)
                nc.vector.tensor_tensor(ch, m1[:], m2[:], ADD)
                tanh_c = wpool.tile([128, HB], BF16, name="g_tc", tag="g_tc", bufs=2)
                nc.scalar.activation(tanh_c[:], ch, TANH)
                nc.vector.tensor_tensor(h_dst, sig_o[:], tanh_c[:], MULT)

            hf1 = wpool.tile([128, IT * H], F32, name="hf1", tag="hf1")
            supP_prev = s0h_sb           # t=-1: host G.h0_init (64 rows)
            stat_cur = spool.tile([128, JT * DIN1], BF16, name="stat1", tag="stat1")
            s1v = stat_cur[:].rearrange("p (jt c) -> p jt c", c=DIN1)
            nc.vector.tensor_copy(
                s1v[:, 0:8, H:DIN1],
                h1i_sb[:].rearrange("p (it c) -> p it c", c=H))

            def e1_mm(psumP, jt, ih, start, stop):
                lhs = s1v[:, jt, :]
                for k in range(K):
                    nc.tensor.matmul(
                        psumP[k][ih][:],
                        lhs,
                        gt_sb[jt][:, k * HALF + ih * 512:
                                  k * HALF + ih * 512 + 512],
                        start=start, stop=stop,
                    )

            def e2_l1_its(t, supP, conv1, its):
                for it in its:
                    pc = ppool.tile([128, DOUT], F32, name="e2p", tag="e2p",
                                    bufs=2)
                    for k in range(K):
                        nc.tensor.matmul(
                            pc[:],
                            supP[k][:, it * 128:(it + 1) * 128],
                            w1_sb[:, (t * K + k) * DOUT:(t * K + k + 1) * DOUT],
                            start=(k == 0), stop=(k == K - 1),
                        )
                    dst = conv1[:, it * DOUT:(it + 1) * DOUT]
                    nc.vector.tensor_tensor(
                        dst, pc[:], b1_sb[:, t * DOUT:(t + 1) * DOUT], ADD)

            for t in range(T):
                # ---------------- layer 0: einsum2 + gates ----------------
                conv0 = wpool.tile([128, IT * DOUT], BF16, name="conv0", tag="conv0")
                for ih in e2_l0(t, supP_prev, conv0):
                    gates(conv0, ih, c_all[0], s1v[:, ih * 4:(ih + 1) * 4, 0:H])
                send_h(s1v[:, 0:8, 0:H], t % 2, 0)
                rs_fire(t % 2, 0)

                # ---------------- einsum1: P_t = G^T [h0_t | h1_{t-1}] ----
                psumP = [[ppool.tile([128, 512], F32, name=f"e1p{k}{ih}",
                                     tag=f"e1p{k}{ih}", bufs=1)
                          for ih in range(2)] for k in range(K)]
                for ih in range(2):
                    for jt in range(8):
                        e1_mm(psumP, jt, ih, jt == 0, False)
                # partner halves arrive at static offsets
                nc.sync.dma_start(
                    s1v[:, 8:16, 0:H],
                    rs_out[0][t % 2].rearrange("p (it c) -> p it c", c=H))
                nc.scalar.dma_start(
                    s1v[:, 8:16, H:DIN1],
                    rs_out[1][t % 2].rearrange("p (it c) -> p it c", c=H))
                supP = [wpool.tile([128, HALF], BF16, name=f"supP{k}",
                                   tag=f"supP{k}", bufs=2) for k in range(K)]
                # partner ih0, evac ih0 (overlaps partner ih1 on tensor)
                for jt in range(8, 16):
                    e1_mm(psumP, jt, 0, False, jt == 15)
                for k in range(K):
                    dst = supP[k][:, 0:512]
                    if k % 2 == 0:
                        nc.vector.tensor_copy(dst, psumP[k][0][:])
                    else:
                        nc.scalar.copy(dst, psumP[k][0][:])
                for jt in range(8, 16):
                    e1_mm(psumP, jt, 1, False, jt == 15)

                # ---------------- layer 1: einsum2 + gates ----------------
                conv1 = wpool.tile([128, IT * DOUT], BF16, name="conv1", tag="conv1")
                e2_l1_its(t, supP, conv1, range(0, 4))
                for k in range(K):
                    dst = supP[k][:, 512:1024]
                    if k % 2 == 0:
                        nc.scalar.copy(dst, psumP[k][1][:])
                    else:
                        nc.vector.tensor_copy(dst, psumP[k][1][:])
                e2_l1_its(t, supP, conv1, range(4, 8))
                if t + 1 < T:
                    stat_next = spool.tile([128, JT * DIN1], BF16, name="stat1",
                                           tag="stat1")
                    s1v_next = stat_next[:].rearrange("p (jt c) -> p jt c", c=DIN1)
                    h1_dst = lambda ih: s1v_next[:, ih * 4:(ih + 1) * 4, H:DIN1]
                else:
                    h1_dst = lambda ih: hf1[:].rearrange(
                        "p (it c) -> p it c", c=H)[:, ih * 4:(ih + 1) * 4, :]
                for ih in range(2):
                    gates(conv1, ih, c_all[1], h1_dst(ih))
                if t + 1 < T:
                    send_h(s1v_next[:, 0:8, H:DIN1], (t + 1) % 2, 1)
                    rs_fire((t + 1) % 2, 1)
                    s1v = s1v_next
                supP_prev = supP

            # ---------------- outputs ----------------
            hf0 = wpool.tile([128, IT * H], F32, name="hf0", tag="hf0")
            nc.vector.tensor_copy(
                hf0[:].rearrange("p (it c) -> p it c", c=H), s1v[:, 0:8, 0:H])
            nc.sync.dma_start(out_ext[0, 0], hf0[:])
            nc.sync.dma_start(out_ext[0, 1], hf1[:])
            nc.sync.dma_start(out_ext[1, 0], c_all[0][:])
            nc.sync.dma_start(out_ext[1, 1], c_all[1][:])

    nc.compile()
    _CACHE["nc"] = nc
    return nc


def _host_prep(inputs):
    """Per-core input maps (all device layouts built here)."""
    G = np.asarray(inputs["G"], np.float32)
    x_seq = np.asarray(inputs["x_seq"], np.float32)
    init_h = np.asarray(inputs["init_h"], np.float32)
    init_c = np.asarray(inputs["init_c"], np.float32)
    x_meta = np.asarray(inputs["x_meta"], np.float32)

    def mlp(b, w1, b1, w2, b2):
        hid = np.maximum(x_meta[b] @ w1 + b1, 0.0)
        return hid @ w2 + b2

    GF = G.reshape(K * N, N)
    in_maps = []
    for c in range(NCORES):
        b, half = c // 2, c % 2
        own = np.arange(half * HALF, (half + 1) * HALF)
        par = np.arange((1 - half) * HALF, (2 - half) * HALF)
        jperm = np.concatenate([own, par])

        # GT[k, j_local, i_own] -> [K, JT, 128, HALF]
        gt = G[:, own, :].transpose(0, 2, 1)[:, jperm, :]
        gt = np.ascontiguousarray(gt.reshape(K, JT, 128, HALF)).astype(BF)

        # host Gx: gxT rows (k,c) at k*C+c, ones row XR-1, zero padding
        xb = np.ascontiguousarray(x_seq[b].transpose(1, 0, 2).reshape(N, T * C))
        gx = (GF @ xb).reshape(K, N, T, C)
        gxt = np.zeros((64, T * HALF), np.float32)
        for k in range(K):
            for cc in range(C):
                gxt[k * C + cc] = gx[k, own, :, cc].T.reshape(T * HALF)
        gxt[XR - 1] = 1.0
        gxt = gxt.astype(BF)

        # host G.h0_init (skip the matmul for the all-zeros init case)
        s0h = np.zeros((K, 64, HALF), np.float32)
        if init_h[0, b].any():
            gh = (GF @ init_h[0, b]).reshape(K, N, H)
            s0h = np.ascontiguousarray(gh[:, own, :].transpose(0, 2, 1))
        s0h = s0h.astype(BF)

        # layer-0 weights: W0 rows [x(0:C) | h(C:C+H)]
        W0 = mlp(b, inputs["lw1_0"], inputs["lb1_0"], inputs["lw2_0"], inputs["lb2_0"])
        W0 = np.asarray(W0, np.float32).reshape(T, K, DIN0, DOUT)
        bias0 = np.asarray(
            mlp(b, inputs["bw1_0"], inputs["bb1_0"], inputs["bw2_0"], inputs["bb2_0"]),
            np.float32)
        w0h = W0[:, :, C:, :].transpose(2, 0, 1, 3).reshape(64, T * K * DOUT).astype(BF)
        w0x = np.zeros((64, T * DOUT), np.float32)
        for k in range(K):
            for cc in range(C):
                w0x[k * C + cc] = W0[:, k, cc, :].reshape(T * DOUT)
        w0x[XR - 1] = bias0.reshape(T * DOUT)
        w0x = w0x.astype(BF)

        W1 = mlp(b, inputs["lw1_1"], inputs["lb1_1"], inputs["lw2_1"], inputs["lb2_1"])
        W1 = np.asarray(W1, np.float32).reshape(T, K, DIN1, DOUT)
        w1 = W1.transpose(2, 0, 1, 3).reshape(DIN1, T * K * DOUT).astype(BF)
        bias1 = np.asarray(
            mlp(b, inputs["bw1_1"], inputs["bb1_1"], inputs["bw2_1"], inputs["bb2_1"]),
            np.float32)
        b1 = np.ascontiguousarray(
            np.broadcast_to(bias1.reshape(1, T * DOUT), (128, T * DOUT))).astype(BF)

        c0 = np.ascontiguousarray(
            init_c[0, b][own].reshape(IT, 128, H).transpose(1, 0, 2).reshape(128, IT * H))
        c1 = np.ascontiguousarray(
            init_c[1, b][own].reshape(IT, 128, H).transpose(1, 0, 2).reshape(128, IT * H))
        h1b = init_h[1, b][own].reshape(IT, 128, H).transpose(1, 0, 2).reshape(
            128, IT * H).astype(BF)

        in_maps.append({
            "gt": gt,
            "sup0h": s0h,
            "w0h": np.ascontiguousarray(w0h),
            "w0x": np.ascontiguousarray(w0x),
            "gxt": np.ascontiguousarray(gxt),
            "w1": np.ascontiguousarray(w1),
            "bias1": b1,
            "c0_init": np.ascontiguousarray(c0, np.float32),
            "c1_init": np.ascontiguousarray(c1, np.float32),
            "h1b_init": np.ascontiguousarray(h1b),
            "mask": np.ascontiguousarray(np.broadcast_to(
                np.array([1 - half, half], np.float32).reshape(1, 2), (128, 2))),
        })
    return in_maps


def kernel(**inputs) -> np.ndarray:
    global LAST_RESULT
    nc = _build()
    in_maps = _host_prep(inputs)
    res = run_bass_kernel_spmd(nc, in_maps, list(range(NCORES)))
    LAST_RESULT = res

    out = np.zeros((2, L, B, N, H), np.float32)
    for c in range(NCORES):
        b, half = c // 2, c % 2
        o = res.results[c]["out"].reshape(2, L, 128, IT, H)
        out[:, :, b, half * HALF:(half + 1) * HALF, :] = o.transpose(0, 1, 3, 2, 4).reshape(
            2, L, HALF, H)
    return out
